# revision 41
# baseline (speedup 1.0000x reference)
"""Distributed causal multi-head attention block for Trainium2 (8 NeuronCores).

Problem: B=4, S=2048, E=1024, H=16 heads, fp32.
    q/k/v = Linear(query/key/value); causal softmax attention; out = Linear(attn).

Sharding: DP=4 over batch x TP=2 over heads. Core c = 2*b + g handles batch b
with heads [8g, 8g+8). Per-core kernel structure (single fused Tile graph):
  - K projection prefix (kT in d-major layout), then a software-pipelined
    merged phase: per q-tile, the next q-tile's V/Q projection tiles and the
    previous q-tile's partial out-projections are interleaved between
    attention heads so TensorE fills the ACT(exp)-bound stretches.
  - Attention is computed in the *transposed* orientation, scoresT[k, q]:
    no max-subtraction (scores are O(1) by construction), no transposes
    anywhere; the softmax denominator comes from an extra ones-column in the
    AV matmul; normalization is a DVE reciprocal + rank-1 broadcast matmul
    whose result lands directly in an SBUF attnT tile (never leaves core).
  - Out-projection via partial sums + ReduceScatter: each core contracts its
    OWN 512 attnT dims against Wo.T[my rows, ALL 1024 columns] (host supplies
    this slice) plus bias/2, then a per-q-tile pairwise ReduceScatter(add)
    sums the two cores' partials and scatters each core its own 512 output
    columns -- written DIRECTLY into the output DRAM tensor. No attnT
    exchange, no gather-dependent matmuls, fully rank-symmetric (SPMD-safe).

DMA discipline: all big inputs ride in ONE bf16 blob laid out so every load
is contiguous per SBUF partition (minimal SWDGE descriptors); xk/w loads on
the SP queue, xq/xv loads on the Activation queue, partial stores +
collectives on the Pool queue. No engine queue ever head-of-line blocks on
another engine's compute.

All matmul accumulation is fp32 in PSUM; error vs the fp32 reference is
bf16 input rounding (~6e-3 relative).
"""
import sys

if "/opt/trn_rl_repo" not in sys.path:
    sys.path.insert(0, "/opt/trn_rl_repo")

import numpy as np

import concourse.bacc as bacc
import concourse.tile as tile
import concourse.mybir as mybir
import concourse.bass_utils as bass_utils

f32 = mybir.dt.float32
bf16 = mybir.dt.bfloat16
Exp = mybir.ActivationFunctionType.Exp

N_CORES = 8
B, S, E = 4, 2048, 1024
H, D = 16, 64
HC = 512            # per-core head dims (8 heads x 64)
SCALE = D ** -0.5
SQ = 512            # q-tile width (columns of scoresT)
SK = 128            # k-chunk (partition rows of scoresT)
NQT = S // SQ       # 4 q-tiles
NE = E // 128       # 8 contraction chunks of the E dim
HS = SQ // 2        # 256-wide half-slabs of the input stream

# blob element offsets (bf16 elements). x streams are stored slab-blocked
# [n, half, p, c, s'], q/k/v weights half-blocked [half, p, c', n], and the
# out-proj weight [p, ic, n_all] so every DMA is contiguous per partition.
_XSZ = E * S
_WSZ = E * HC
_SLAB = 128 * NE * HS          # one x half-slab
_WHALF = 128 * (NE // 2) * HC  # one q/k/v weight half
OFF_XQ = 0
OFF_XK = OFF_XQ + _XSZ
OFF_XV = OFF_XK + _XSZ
OFF_WQ = OFF_XV + _XSZ
OFF_WK = OFF_WQ + _WSZ
OFF_WV = OFF_WK + _WSZ
OFF_WO = OFF_WV + _WSZ                # [p, ic, 2*HC] = my rows x all cols
OFF_BIASES = OFF_WO + 128 * 4 * 2 * HC  # [1, 2048]: bv | bo_all/2 | pad
OFF_MASKS = OFF_BIASES + 4 * HC       # [128, 4, SQ]
OFF_ONES = OFF_MASKS + 128 * 4 * SQ   # [65, SQ]
OFF_BCOL = OFF_ONES + 65 * SQ         # [128, 8]: bq[m] (j=m) / bk[m] (4+m)
BLOB_LEN = OFF_BCOL + 128 * 8


def build_nc(skip_cc=False, lag=2, eager=10, pad_elems=0):
    nc = bacc.Bacc("TRN2", target_bir_lowering=False, debug=False,
                   num_devices=N_CORES)

    blob = nc.declare_dram_parameter("blob", [BLOB_LEN], bf16, isOutput=False)
    if pad_elems:
        nc.declare_dram_parameter("pad", [pad_elems], bf16, isOutput=False)
    # out rows are [qt, p, mm] blocked (s = qt*SQ + mm*128 + p); the host
    # unpermutes. This keeps every rsin/rsout/out DMA fully contiguous.
    out = nc.declare_dram_parameter("out", [NQT, 128, 4, HC], bf16,
                                    isOutput=True)

    def bview(off, pattern, **sizes):
        import re
        names = re.findall(r"\w+", pattern.split("->")[0])
        length = 1
        for n in names:
            length *= sizes[n]
        return blob.ap()[off:off + length].rearrange(
            f"({' '.join(names)}) -> {pattern.split('->')[1].strip()}", **sizes)

    def slab_view(stream, n, half):
        base = AGOFF[stream]
        return bview(base + (n * 2 + half) * _SLAB,
                     "p c s -> p c s", p=128, c=NE, s=HS)

    def whalf_view(base, half):
        return bview(base + half * _WHALF,
                     "p c n -> p c n", p=128, c=NE // 2, n=HC)

    wo_ap = bview(OFF_WO, "p c n -> p c n", p=128, c=4, n=2 * HC)
    biases_ap = bview(OFF_BIASES, "r n -> r n", r=1, n=4 * HC)
    masks_ap = bview(OFF_MASKS, "p r q -> p r q", p=128, r=4, q=SQ)
    ones_ap = bview(OFF_ONES, "a q -> a q", a=65, q=SQ)
    bcol_ap = bview(OFF_BCOL, "p j -> p j", p=128, j=8)

    # ReduceScatter staging: per q-tile, my partial out-proj for BOTH cores'
    # output columns, j-major [j, p, mm, e] so block j is one contiguous
    # half; the collective sums pairs and scatters block g to core g.
    # Two collectives per q-tile (mm 0-1 / mm 2-3) so the first fires while
    # the second half's partials are still on TensorE.
    rsin = [[nc.dram_tensor(f"rsin{i}_{h}", [2, 128, 2, HC], bf16)
             for h in range(2)] for i in range(4)]
    rsout = [[nc.dram_tensor(f"rsout{i}_{h}", [128, 2, HC], bf16)
              for h in range(2)] for i in range(4)]
    RG = [[0, 1], [2, 3], [4, 5], [6, 7]]

    AGOFF = {"q": OFF_XQ, "k": OFF_XK, "v": OFF_XV}

    with tile.TileContext(nc) as tc:
        with tc.tile_pool(name="persist", bufs=1) as pp, \
             tc.tile_pool(name="xsp", bufs=4) as xsp, \
             tc.tile_pool(name="qtp", bufs=2) as qtp, \
             tc.tile_pool(name="atp", bufs=3) as atp, \
             tc.tile_pool(name="att", bufs=lag + 3) as att, \
             tc.tile_pool(name="attr", bufs=5) as attr, \
             tc.tile_pool(name="opo", bufs=2) as opo, \
             tc.tile_pool(name="psA", bufs=2, space="PSUM") as psA, \
             tc.tile_pool(name="psS", bufs=lag + 1, space="PSUM") as psS, \
             tc.tile_pool(name="psAV", bufs=2, space="PSUM") as psAV, \
             tc.tile_pool(name="psB", bufs=1, space="PSUM") as psB:
            kT = pp.tile([128, 4, S], bf16)       # [p, m, s]: k-dim = m*128+p
            v4 = pp.tile([128, 16, 8, 65], bf16)  # [p, sc, h, j]: v row sc*128+p
            masks_t = pp.tile([128, 4, SQ], bf16)
            ones_t = pp.tile([65, SQ], bf16)
            bt = pp.tile([65, 4 * HC], bf16)  # p64: bv | bo_all/2 | pad
            bc_raw = pp.tile([128, 8], bf16)
            bc_t = pp.tile([128, 8], f32)     # col j: bq[m] (j=m) / bk[m] (4+m)
            bias_bc = pp.tile([128, 2, HC], bf16)  # bo_half bcast to 128 rows
            wq_t = pp.tile([128, NE, HC], bf16)
            wv_t = pp.tile([128, NE, HC], bf16)
            # wk lives in its own pool: its slot is handed to wo mid-loop,
            # after the last kT tile is produced
            wkp_cm = tc.tile_pool(name="wkp", bufs=1)
            wkp = wkp_cm.__enter__()
            wk_t = wkp.tile([128, NE, HC], bf16)
            holder = {}

            def dma_w_half(dst, base, i, split=False):
                half = NE // 2
                src_ap = whalf_view(base, i)
                if split:
                    for kc in range(half):
                        nc.sync.dma_start(out=dst[:, i * half + kc, :],
                                          in_=src_ap[:, kc, :])
                else:
                    nc.sync.dma_start(
                        out=dst[:, i * half:(i + 1) * half, :], in_=src_ap)

            def dma_w(dst, base):
                dma_w_half(dst, base, 0)
                dma_w_half(dst, base, 1)

            def load_half(stream, n, half, split=False, eng=None):
                eng = eng or nc.sync
                xs = xsp.tile([128, NE, HS], bf16, tag="x")
                src_ap = slab_view(stream, n, half)
                if split:
                    for kc in range(NE):
                        eng.dma_start(out=xs[:, kc, :], in_=src_ap[:, kc, :])
                else:
                    eng.dma_start(out=xs[:], in_=src_ap)
                return xs

            # both wk halves first (the first qk_tile accumulates over all 8
            # chunks); the first half per-chunk so matmul kc=0 starts as soon
            # as 128KB (not 512KB) has landed. Small tiles ride gpsimd.
            nc.gpsimd.dma_start(out=bc_raw[:], in_=bcol_ap)
            nc.gpsimd.dma_start(out=ones_t[:], in_=ones_ap)
            nc.gpsimd.dma_start(out=bt[64:65, :], in_=biases_ap[0:1, :])
            nc.vector.tensor_copy(bc_t[:], bc_raw[:])  # bf16 -> f32 scalars
            dma_w_half(wk_t, OFF_WK, 0, split=True)
            dma_w_half(wk_t, OFF_WK, 1)

            def qk_tile(dst_ap_fn, w_t, bj, xs, m):
                # one [128, HS] output tile of a q/k-style projection;
                # bias is folded into the PSUM->SBUF copy (per-partition add)
                ps = psA.tile([128, HS], f32, tag="pp")
                for kc in range(NE):
                    nc.tensor.matmul(ps[:], w_t[:, kc, m * 128:(m + 1) * 128],
                                     xs[:, kc, :], start=(kc == 0),
                                     stop=(kc == NE - 1))
                nc.vector.tensor_scalar(dst_ap_fn(), ps[:],
                                        bc_t[:, bj + m:bj + m + 1], None,
                                        op0=mybir.AluOpType.add)

            def v_tile(xs, sc, mm):
                # one [128 S-rows, 512 v-dims] tile of the V projection.
                # bv is folded into the out-proj bias on the host (softmax
                # rows sum to 1, so attn@(Vx+bv) == attn@Vx + bv): no bias
                # matmul here.
                ps = psA.tile([128, HC], f32, tag="pp")
                for kc in range(NE):
                    nc.tensor.matmul(ps[:], xs[:, kc, mm * 128:(mm + 1) * 128],
                                     wv_t[:, kc, :], start=(kc == 0),
                                     stop=(kc == NE - 1))
                nc.vector.tensor_copy(
                    v4[:, sc, :, 0:64],
                    ps[:].rearrange("p (h j) -> p h j", h=8))

            qtiles = [None] * NQT
            atps = [None] * NQT
            xv_cur = [None]
            xq_cur = [None]
            pp_done = [0] * NQT

            def proj_tasks(n):
                # v-slab n + q-slab n as resumable tile tasks
                qtiles[n] = qtp.tile([128, 4, SQ], bf16, tag="qt",
                                     name=f"qtile{n}")
                tasks = []
                for half in range(2):
                    for mm in range(2):
                        tasks.append(("v", n, mm, half, 0))
                    for m in range(4):
                        tasks.append(("q", n, m, half, 0))
                return tasks

            def partial_tile(part, j, mm):
                # my-rows contraction for output block j (rank j's columns),
                # s-rows mm; + bias/2 (DVE add of the pre-broadcast bias
                # tile -- no bias matmul). Summed across the pair by the
                # ReduceScatter that fires after the 8th tile.
                at_t = atps[part]
                wo_t = holder["wo_t"]
                po = psA.tile([128, HC], f32, tag="pp")
                for ic in range(4):
                    nc.tensor.matmul(po[:], at_t[:, ic, mm * 128:(mm + 1) * 128],
                                     wo_t[:, ic, j * HC:(j + 1) * HC],
                                     start=(ic == 0), stop=(ic == 3))
                okey = ("ob", part)
                if okey not in holder:
                    holder[okey] = opo.tile([128, 2, 4, HC], bf16, tag="ot",
                                            name=f"ob{part}")
                ob = holder[okey]
                nc.vector.tensor_tensor(ob[:, j, mm, :], po[:],
                                        bias_bc[:, j, :],
                                        op=mybir.AluOpType.add)
                pp_done[part] += 1
                if pp_done[part] in (4, 8):
                    # contiguous stores on Pool so the wait for the partials
                    # doesn't head-of-line block a load queue -- only the
                    # collective rides behind them anyway
                    h2 = pp_done[part] // 4 - 1
                    sl = slice(2 * h2, 2 * h2 + 2)
                    for j2 in range(2):
                        nc.gpsimd.dma_start(out=rsin[part][h2].ap()[j2],
                                            in_=ob[:, j2, sl, :])
                    if skip_cc:
                        nc.sync.dma_start(out=out.ap()[part][:, sl, :],
                                          in_=rsin[part][h2].ap()[0])
                    else:
                        # collectives cannot write IO tensors: scatter into
                        # scratch, then DRAM->DRAM DMA into the output
                        nc.gpsimd.collective_compute(
                            "ReduceScatter", mybir.AluOpType.add,
                            replica_groups=RG,
                            ins=[rsin[part][h2].ap().opt()],
                            outs=[rsout[part][h2].ap().opt()])
                        nc.sync.dma_start(out=out.ap()[part][:, sl, :],
                                          in_=rsout[part][h2].ap())

            pre = {}

            def run_task(t):
                kind, n, m, half = t[0], t[1], t[2], t[3]
                if kind == "v":
                    if m == 0:
                        xv_cur[0] = pre.pop(("v", n, half), None) or \
                            load_half("v", n, half, eng=nc.scalar)
                    v_tile(xv_cur[0], n * 4 + half * 2 + m, m)
                elif kind == "q":
                    if m == 0:
                        xq_cur[0] = pre.pop(("q", n, half), None) or \
                            load_half("q", n, half, eng=nc.scalar)
                    qtl = qtiles[n]
                    qk_tile(lambda: qtl[:, m, half * HS:(half + 1) * HS],
                            wq_t, 0, xq_cur[0], m)
                else:
                    partial_tile(n, m, half)

            # ---------------- prefix: full K projection ----------------
            # alternate slab loads across both HWDGE queues (scalar is idle
            # until the merged phase) so DMA arrival outruns PE consumption
            for n in range(4):
                for half in range(2):
                    first = n == 0 and half == 0
                    xs = load_half("k", n, half, split=first,
                                   eng=nc.scalar if (n * 2 + half) % 2 == 0
                                   else nc.sync)
                    if n == 0 and half == 1:
                        # masks/vones are not consumed until attention:
                        # keep them off the critical first-slab DMA window
                        nc.gpsimd.dma_start(out=masks_t[:], in_=masks_ap)
                        nc.gpsimd.memset(v4[:, :, :, 64], 1.0)
                    if n == 2 and half == 0:
                        # v/q weights land well before the merged phase
                        dma_w(wv_t, OFF_WV)
                        dma_w(wq_t, OFF_WQ)
                    if n == 3 and half == 1:
                        # prefetch the first merged-phase x slabs so the
                        # proj_tasks(0) stretch doesn't stall on the ACT
                        # queue's first transfer
                        pre[("v", 0, 0)] = load_half("v", 0, 0,
                                                     eng=nc.scalar)
                        pre[("q", 0, 0)] = load_half("q", 0, 0,
                                                     eng=nc.scalar)
                    for m in range(4):
                        off = n * SQ + half * HS
                        qk_tile(lambda m=m, off=off: kT[:, m, off:off + HS],
                                wk_t, 4, xs, m)
            # broadcast bo_half to all 128 partitions once (replaces the
            # per-partial-tile bias matmul)
            for j in range(2):
                pb0 = psA.tile([128, HC], f32, tag="pp")
                nc.tensor.matmul(pb0[:], ones_t[64:65, 0:128],
                                 bt[64:65, HC + j * HC:HC + (j + 1) * HC],
                                 start=True, stop=True)
                nc.vector.tensor_copy(bias_bc[:, j, :], pb0[:])

            # wk's SBUF slot is handed to wo_t; attention pools open here
            wkp_cm.__exit__(None, None, None)
            wop_cm = tc.tile_pool(name="wop", bufs=1)
            wop = wop_cm.__enter__()
            wo_t = wop.tile([128, 4, 2 * HC], bf16)
            holder["wo_t"] = wo_t
            holder["cm"] = wop_cm

            # ---------------- merged v/q projections + attention ----------
            for t in proj_tasks(0):
                run_task(t)
            nc.sync.dma_start(out=wo_t[:, 0:2, :], in_=wo_ap[:, 0:2, :])
            nc.sync.dma_start(out=wo_t[:, 2:4, :], in_=wo_ap[:, 2:4, :])

            work = []
            pending_fin = None
            for qt in range(NQT):
                if qt > 0:
                    # partial out-proj of the previous q-tile: local data
                    # only, safe to interleave from h0. mm-major so the
                    # first-half ReduceScatter fires after 4 tiles.
                    work.extend(("pp", qt - 1, j, mm, 0)
                                for mm in range(4) for j in range(2))
                if qt + 1 < NQT:
                    work.extend(proj_tasks(qt + 1))
                atps[qt] = atp.tile([128, 4, SQ], bf16, tag="at",
                                    name=f"atp{qt}")
                for h in range(8):
                    m, po = h // 2, 64 * (h % 2)
                    pav = psAV.tile([65, SQ], f32, tag="av")
                    nkc = (qt + 1) * (SQ // SK)
                    pts = {}
                    qtl = qtiles[qt]

                    def issue_score(kc, qt=qt, m=m, po=po, pts=pts, qtl=qtl):
                        r = kc - 4 * qt
                        # diagonal blocks: columns < r*128 are fully masked;
                        # restrict the score matmul too
                        s0 = r * SK if r in (1, 2, 3) else 0
                        pscore = psS.tile([128, SQ], f32, tag="sc")
                        nc.tensor.matmul(
                            pscore[:, s0:],
                            kT[po:po + 64, m, kc * SK:(kc + 1) * SK],
                            qtl[po:po + 64, m, s0:],
                            start=True, stop=True)
                        pt = att.tile([128, SQ], bf16, tag="pt")
                        if 0 <= r <= 3:
                            # columns < r*128 are fully masked: skip them
                            c0 = r * SK
                            praw = attr.tile([128, SQ], bf16, tag="praw")
                            nc.scalar.activation(praw[:, c0:], pscore[:, c0:],
                                                 Exp, scale=SCALE)
                            nc.vector.tensor_tensor(
                                pt[:, c0:], praw[:, c0:], masks_t[:, r, c0:],
                                op=mybir.AluOpType.mult)
                            pts[kc] = (pt, c0)
                        else:
                            nc.scalar.activation(pt[:], pscore[:], Exp,
                                                 scale=SCALE)
                            pts[kc] = (pt, 0)

                    for kc in range(min(lag, nkc)):
                        issue_score(kc)
                    # finalize the previous head while this head's score
                    # pipeline fills, so TensorE never waits on the DVE
                    # reciprocal chain
                    if pending_fin is not None:
                        pending_fin()
                        pending_fin = None
                    for kc in range(nkc):
                        if kc + lag < nkc:
                            issue_score(kc + lag)
                        pt, c0 = pts.pop(kc)
                        # kc==0 always has c0==0, so start covers the whole
                        # [65, 512] accumulator
                        nc.tensor.matmul(pav[:, c0:], v4[:, kc, h, :],
                                         pt[:, c0:],
                                         start=(kc == 0), stop=(kc == nkc - 1))

                    def finalize(qt=qt, h=h, pav=pav):
                        # normalized attnT rows land straight in the SBUF
                        # atp tile -- attnT never leaves the core
                        rt = attr.tile([1, SQ], bf16, tag="rt")
                        with nc.allow_low_precision(reason="bf16 pipeline"):
                            nc.vector.reciprocal(rt[:], pav[64:65, :])
                        pb = psB.tile([64, SQ], f32, tag="bc")
                        nc.tensor.matmul(pb[:], ones_t[0:1, 0:64], rt[:],
                                         start=True, stop=True)
                        pbs = attr.tile([64, SQ], bf16, tag="pbs")
                        nc.vector.tensor_copy(pbs[:], pb[:])
                        p0 = 64 * (h % 2)
                        nc.vector.tensor_tensor(
                            atps[qt][p0:p0 + 64, h // 2, :],
                            pav[0:64, :], pbs[:], op=mybir.AluOpType.mult)

                    pending_fin = finalize
                    ready = sum(1 for t in work if t[4] <= h)
                    npop = -(-ready // max(1, eager - h)) if ready else 0
                    for _ in range(npop):
                        run_task(work.pop(0))
                if pending_fin is not None:
                    pending_fin()
                    pending_fin = None
                while work:
                    run_task(work.pop(0))
            # last q-tile's partial out-proj + its ReduceScatter
            for mm in range(4):
                for j in range(2):
                    partial_tile(3, j, mm)
            holder["cm"].__exit__(None, None, None)

    nc.compile()
    return nc


_NC_CACHE = None


def _get_nc():
    global _NC_CACHE
    if _NC_CACHE is None:
        _NC_CACHE = build_nc()
    return _NC_CACHE


def _prep_x(xT):
    # [E, S] -> slab-blocked [n, half, p, c, s'] (contiguous per partition)
    a = xT.reshape(NE, 128, NQT, 2, HS)          # (c, p, n, half, s')
    return np.ascontiguousarray(a.transpose(2, 3, 1, 0, 4))


def _prep_w(wT):
    # [E, HC] -> half-blocked [half, p, c', n]
    a = wT.reshape(2, NE // 2, 128, HC)          # (half, c', p, n)
    return np.ascontiguousarray(a.transpose(0, 2, 1, 3))


def _prepare_in_maps(query, key, value, Wq, bq, Wk, bk, Wv, bv, Wo, bo):
    import ml_dtypes
    bfl = ml_dtypes.bfloat16

    query = np.asarray(query, dtype=np.float32)
    key = np.asarray(key, dtype=np.float32)
    value = np.asarray(value, dtype=np.float32)

    xT = [[_prep_x(np.ascontiguousarray(a[b].T).astype(bfl)) for b in range(B)]
          for a in (query, key, value)]

    p = np.arange(128)[:, None, None]
    r = np.arange(4)[None, :, None]
    qn = np.arange(SQ)[None, None, :]
    masks = ((p + r * 128) <= qn).astype(bfl)
    ones = np.ones((65, SQ), dtype=bfl)

    WoT = np.ascontiguousarray(np.asarray(Wo).T)   # [E, 1024] all columns
    # softmax rows sum to 1, so attn@(Vx+bv) == attn@Vx + bv: fold bv into
    # the out-proj bias (exact)
    bo_fold = (np.asarray(bv, dtype=np.float64) @ np.asarray(WoT, np.float64)
               + np.asarray(bo, np.float64)).astype(np.float32)
    bo_half = (bo_fold * 0.5).astype(np.float32)

    w_g, biases_g, bcol_g, wo_g = [], [], [], []
    for g in range(2):
        sl = slice(g * HC, (g + 1) * HC)
        ws = [_prep_w(np.ascontiguousarray(np.asarray(W)[sl, :].T).astype(bfl))
              for W in (Wq, Wk, Wv)]
        w_g.append(ws)
        # out-proj: MY 512 rows (= my heads' dims) x ALL 1024 columns,
        # [p, ic, n] blocked
        wo_my = WoT[g * HC:(g + 1) * HC, :].reshape(4, 128, 2 * HC)
        wo_g.append(np.ascontiguousarray(
            wo_my.transpose(1, 0, 2)).astype(bfl))
        biases_g.append(np.concatenate([
            np.zeros(HC, np.float32), bo_half,
            np.zeros(HC, np.float32)]).reshape(1, 4 * HC).astype(bfl))
        bc = np.empty((128, 8), dtype=np.float32)
        for mj in range(4):
            bc[:, mj] = np.asarray(bq)[g * HC + mj * 128:g * HC + (mj + 1) * 128]
            bc[:, 4 + mj] = np.asarray(bk)[g * HC + mj * 128:g * HC + (mj + 1) * 128]
        bcol_g.append(bc)

    in_maps = []
    blob_cache = {}
    for c in range(N_CORES):
        b, g = c // 2, c % 2
        key_ = (b, g)
        if key_ not in blob_cache:
            parts = [xT[0][b], xT[1][b], xT[2][b]] + w_g[g] + \
                    [wo_g[g], biases_g[g], masks, ones,
                     bcol_g[g].astype(bfl)]
            blob = np.concatenate([np.ascontiguousarray(a).reshape(-1)
                                   for a in parts])
            assert blob.shape[0] == BLOB_LEN, (blob.shape, BLOB_LEN)
            blob_cache[key_] = blob
        in_maps.append({"blob": blob_cache[key_]})
    return in_maps


def run(trace=False, **inputs):
    in_maps = _prepare_in_maps(**inputs)
    nc = _get_nc()
    res = bass_utils.run_bass_kernel_spmd(
        nc, in_maps, core_ids=list(range(N_CORES)), trace=trace)
    full = np.empty((B, S, E), dtype=np.float32)
    for c in range(N_CORES):
        b, g = c // 2, c % 2
        # out rows are [qt, p, mm] blocked: s = qt*SQ + mm*128 + p
        o = res.results[c]["out"].astype(np.float32)
        full[b, :, g * HC:(g + 1) * HC] = o.transpose(0, 2, 1, 3).reshape(
            S, HC)
    return full, res


def kernel(**inputs) -> np.ndarray:
    full, _ = run(trace=False, **inputs)
    return full


def bench(n_iters=5, repeats=5, nc=None, **inputs):
    """Estimate on-device NEFF time: chain n_iters executions with a tiny
    data dependency (no CSE, strict serialization), time with device-resident
    inputs, and report the marginal per-iteration wall time."""
    import time
    import jax
    from jax.sharding import Mesh, PartitionSpec
    from jax.experimental.shard_map import shard_map
    import concourse.bass2jax as bass2jax
    import concourse.mybir as mb

    if nc is None:
        nc = _get_nc()
    in_maps = _prepare_in_maps(**inputs)
    bass2jax.install_neuronx_cc_hook()

    partition_name = nc.partition_id_tensor.name if nc.partition_id_tensor else None
    in_names, out_names, out_avals = [], [], []
    for alloc in nc.m.functions[0].allocations:
        if not isinstance(alloc, mb.MemoryLocationSet):
            continue
        name = alloc.memorylocations[0].name
        if alloc.kind == "ExternalInput":
            if name != partition_name:
                in_names.append(name)
        elif alloc.kind == "ExternalOutput":
            out_names.append(name)
            out_avals.append(
                jax.core.ShapedArray(tuple(alloc.tensor_shape),
                                     mb.dt.np(alloc.dtype)))
    n_params = len(in_names)
    all_in_names = list(in_names) + list(out_names)
    if partition_name is not None:
        all_in_names.append(partition_name)

    def _body(*args):
        operands = list(args)
        if partition_name is not None:
            operands.append(bass2jax.partition_id_tensor())
        outs = bass2jax._bass_exec_p.bind(
            *operands,
            out_avals=tuple(out_avals),
            in_names=tuple(all_in_names),
            out_names=tuple(out_names),
            lowering_input_output_aliases=(),
            sim_require_finite=True,
            sim_require_nnan=True,
            nc=nc)
        return tuple(outs)

    devices = jax.devices()[:N_CORES]
    mesh = Mesh(np.asarray(devices), ("core",))
    n_outs = len(out_names)
    in_specs = (PartitionSpec("core"),) * (n_params + n_outs)
    out_specs = (PartitionSpec("core"),) * n_outs

    per_core = [[np.asarray(m[name]) for name in in_names] for m in in_maps]
    concat_in = [np.concatenate([per_core[c][i] for c in range(N_CORES)], axis=0)
                 for i in range(n_params)]
    concat_zeros = [np.zeros((N_CORES * a.shape[0], *a.shape[1:]), a.dtype)
                    for a in out_avals]

    sharding = jax.sharding.NamedSharding(mesh, PartitionSpec("core"))
    dev_in = [jax.device_put(x, sharding) for x in concat_in + concat_zeros]

    # donate the output buffers and chain each call's outputs into the next
    # call's donated outputs: executions serialize on-device, memory stays
    # bounded, and M iterations aggregate enough device time to dominate the
    # ~100ms axon RTT quantum.
    donate = tuple(range(n_params, n_params + n_outs))
    fn = jax.jit(shard_map(_body, mesh=mesh, in_specs=in_specs,
                           out_specs=out_specs, check_rep=False),
                 keep_unused=True, donate_argnums=donate)
    params = dev_in[:n_params]
    outs = tuple(dev_in[n_params:])
    outs = fn(*params, *outs)  # warm
    jax.block_until_ready(outs)

    def run_m(m):
        nonlocal outs
        t0 = time.perf_counter()
        for _ in range(m):
            outs = fn(*params, *outs)
        jax.block_until_ready(outs)
        return time.perf_counter() - t0

    m_lo, m_hi = 8, 8 + n_iters
    t_lo = min(run_m(m_lo) for _ in range(repeats))
    t_hi = min(run_m(m_hi) for _ in range(repeats))
    marginal = (t_hi - t_lo) / (m_hi - m_lo)
    return marginal * 1e9, {"m_lo": (m_lo, t_lo), "m_hi": (m_hi, t_hi)}



# revision 47
# speedup vs baseline: 1.7378x; 1.7378x over previous
"""Distributed causal multi-head attention block for Trainium2 (8 NeuronCores).

Problem: B=4, S=2048, E=1024, H=16 heads, fp32.
    q/k/v = Linear(query/key/value); causal softmax attention; out = Linear(attn).

Sharding: DP=4 over batch x TP=2 over heads. Core c = 2*b + g handles batch b
with heads [8g, 8g+8). Per-core kernel structure (single fused Tile graph):
  - K projection prefix (kT in d-major layout), then a software-pipelined
    merged phase: per q-tile, the next q-tile's V/Q projection tiles and the
    previous q-tile's partial out-projections are interleaved between
    attention heads so TensorE fills the ACT(exp)-bound stretches.
  - Attention is computed in the *transposed* orientation, scoresT[k, q]:
    no max-subtraction (scores are O(1) by construction), no transposes
    anywhere; the softmax denominator comes from an extra ones-column in the
    AV matmul; normalization is a DVE reciprocal + rank-1 broadcast matmul
    whose result lands directly in an SBUF attnT tile (never leaves core).
  - Out-projection via partial sums + ReduceScatter: each core contracts its
    OWN 512 attnT dims against Wo.T[my rows, ALL 1024 columns] (host supplies
    this slice) plus bias/2, then a per-q-tile pairwise ReduceScatter(add)
    sums the two cores' partials and scatters each core its own 512 output
    columns -- written DIRECTLY into the output DRAM tensor. No attnT
    exchange, no gather-dependent matmuls, fully rank-symmetric (SPMD-safe).

DMA discipline: all big inputs ride in ONE bf16 blob laid out so every load
is contiguous per SBUF partition (minimal SWDGE descriptors); xk/w loads on
the SP queue, xq/xv loads on the Activation queue, partial stores +
collectives on the Pool queue. No engine queue ever head-of-line blocks on
another engine's compute.

All matmul accumulation is fp32 in PSUM; error vs the fp32 reference is
bf16 input rounding (~6e-3 relative).
"""
import sys

if "/opt/trn_rl_repo" not in sys.path:
    sys.path.insert(0, "/opt/trn_rl_repo")

import numpy as np

import concourse.bacc as bacc
import concourse.tile as tile
import concourse.mybir as mybir
import concourse.bass_utils as bass_utils

f32 = mybir.dt.float32
bf16 = mybir.dt.bfloat16
Exp = mybir.ActivationFunctionType.Exp

N_CORES = 8
B, S, E = 4, 2048, 1024
H, D = 16, 64
HC = 512            # per-core head dims (8 heads x 64)
SCALE = D ** -0.5
SQ = 512            # q-tile width (columns of scoresT)
SK = 128            # k-chunk (partition rows of scoresT)
NQT = S // SQ       # 4 q-tiles
NE = E // 128       # 8 contraction chunks of the E dim
HS = SQ // 2        # 256-wide half-slabs of the input stream

# blob element offsets (bf16 elements). x streams are stored slab-blocked
# [n, half, p, c, s'], q/k/v weights half-blocked [half, p, c', n], and the
# out-proj weight [p, ic, n_all] so every DMA is contiguous per partition.
_XSZ = E * S
_WSZ = E * HC
_SLAB = 128 * NE * HS          # one x half-slab
_WHALF = 128 * (NE // 2) * HC  # one q/k/v weight half
OFF_XQ = 0
OFF_XK = OFF_XQ + _XSZ
OFF_XV = OFF_XK + _XSZ
OFF_WQ = OFF_XV + _XSZ
OFF_WK = OFF_WQ + _WSZ
OFF_WV = OFF_WK + _WSZ
OFF_WO = OFF_WV + _WSZ                # [p, ic, 2*HC] = my rows x all cols
OFF_BIASES = OFF_WO + 128 * 4 * 2 * HC  # [1, 2048]: bv | bo_all/2 | pad
OFF_MASKS = OFF_BIASES + 4 * HC       # [128, 4, SQ]
OFF_ONES = OFF_MASKS + 128 * 4 * SQ   # [65, SQ]
OFF_BCOL = OFF_ONES + 65 * SQ         # [128, 8]: bq[m] (j=m) / bk[m] (4+m)
BLOB_LEN = OFF_BCOL + 128 * 8


def build_nc(skip_cc=False, lag=3, eager=10, pad_elems=0):
    nc = bacc.Bacc("TRN2", target_bir_lowering=False, debug=False,
                   num_devices=N_CORES)

    blob = nc.declare_dram_parameter("blob", [BLOB_LEN], bf16, isOutput=False)
    if pad_elems:
        nc.declare_dram_parameter("pad", [pad_elems], bf16, isOutput=False)
    # out rows are [qt, p, mm] blocked (s = qt*SQ + mm*128 + p); the host
    # unpermutes. This keeps every rsin/rsout/out DMA fully contiguous.
    out = nc.declare_dram_parameter("out", [NQT, 128, 4, HC], bf16,
                                    isOutput=True)

    def bview(off, pattern, **sizes):
        import re
        names = re.findall(r"\w+", pattern.split("->")[0])
        length = 1
        for n in names:
            length *= sizes[n]
        return blob.ap()[off:off + length].rearrange(
            f"({' '.join(names)}) -> {pattern.split('->')[1].strip()}", **sizes)

    def slab_view(stream, n, half):
        base = AGOFF[stream]
        return bview(base + (n * 2 + half) * _SLAB,
                     "p c s -> p c s", p=128, c=NE, s=HS)

    def whalf_view(base, half):
        return bview(base + half * _WHALF,
                     "p c n -> p c n", p=128, c=NE // 2, n=HC)

    wo_ap = bview(OFF_WO, "p c n -> p c n", p=128, c=4, n=2 * HC)
    biases_ap = bview(OFF_BIASES, "r n -> r n", r=1, n=4 * HC)
    masks_ap = bview(OFF_MASKS, "p r q -> p r q", p=128, r=4, q=SQ)
    ones_ap = bview(OFF_ONES, "a q -> a q", a=65, q=SQ)
    bcol_ap = bview(OFF_BCOL, "p j -> p j", p=128, j=8)

    # ReduceScatter staging: per q-tile, my partial out-proj for BOTH cores'
    # output columns, j-major [j, p, mm, e] so block j is one contiguous
    # half; the collective sums pairs and scatters block g to core g.
    # Two collectives per q-tile (mm 0-1 / mm 2-3) so the first fires while
    # the second half's partials are still on TensorE.
    rsin = [[nc.dram_tensor(f"rsin{i}_{h}", [2, 128, 2, HC], bf16)
             for h in range(2)] for i in range(4)]
    rsout = [[nc.dram_tensor(f"rsout{i}_{h}", [128, 2, HC], bf16)
              for h in range(2)] for i in range(4)]
    RG = [[0, 1], [2, 3], [4, 5], [6, 7]]

    AGOFF = {"q": OFF_XQ, "k": OFF_XK, "v": OFF_XV}

    with tile.TileContext(nc) as tc:
        with tc.tile_pool(name="persist", bufs=1) as pp, \
             tc.tile_pool(name="xsp", bufs=4) as xsp, \
             tc.tile_pool(name="qtp", bufs=2) as qtp, \
             tc.tile_pool(name="atp", bufs=3) as atp, \
             tc.tile_pool(name="att", bufs=lag + 3) as att, \
             tc.tile_pool(name="attr", bufs=5) as attr, \
             tc.tile_pool(name="opo", bufs=2) as opo, \
             tc.tile_pool(name="psA", bufs=2, space="PSUM") as psA, \
             tc.tile_pool(name="psS", bufs=lag + 1, space="PSUM") as psS, \
             tc.tile_pool(name="psAV", bufs=2, space="PSUM") as psAV:
            kT = pp.tile([128, 4, S], bf16)       # [p, m, s]: k-dim = m*128+p
            v4 = pp.tile([128, 16, 8, 65], bf16)  # [p, sc, h, j]: v row sc*128+p
            masks_t = pp.tile([128, 4, SQ], bf16)
            ones_t = pp.tile([65, SQ], bf16)
            bt = pp.tile([65, 4 * HC], bf16)  # p64: bv | bo_all/2 | pad
            bc_raw = pp.tile([128, 8], bf16)
            bc_t = pp.tile([128, 8], f32)     # col j: bq[m] (j=m) / bk[m] (4+m)
            bias_bc = pp.tile([128, 2, HC], bf16)  # bo_half bcast to 128 rows
            wq_t = pp.tile([128, NE, HC], bf16)
            wv_t = pp.tile([128, NE, HC], bf16)
            # wk lives in its own pool: its slot is handed to wo mid-loop,
            # after the last kT tile is produced
            wkp_cm = tc.tile_pool(name="wkp", bufs=1)
            wkp = wkp_cm.__enter__()
            wk_t = wkp.tile([128, NE, HC], bf16)
            holder = {}

            def dma_w_half(dst, base, i, split=False):
                half = NE // 2
                src_ap = whalf_view(base, i)
                if split:
                    for kc in range(half):
                        nc.sync.dma_start(out=dst[:, i * half + kc, :],
                                          in_=src_ap[:, kc, :])
                else:
                    nc.sync.dma_start(
                        out=dst[:, i * half:(i + 1) * half, :], in_=src_ap)

            def dma_w(dst, base):
                dma_w_half(dst, base, 0)
                dma_w_half(dst, base, 1)

            def load_half(stream, n, half, split=False, eng=None):
                eng = eng or nc.sync
                xs = xsp.tile([128, NE, HS], bf16, tag="x")
                src_ap = slab_view(stream, n, half)
                if split:
                    for kc in range(NE):
                        eng.dma_start(out=xs[:, kc, :], in_=src_ap[:, kc, :])
                else:
                    eng.dma_start(out=xs[:], in_=src_ap)
                return xs

            # both wk halves first (the first qk_tile accumulates over all 8
            # chunks); the first half per-chunk so matmul kc=0 starts as soon
            # as 128KB (not 512KB) has landed. Small tiles ride gpsimd.
            nc.gpsimd.dma_start(out=bc_raw[:], in_=bcol_ap)
            nc.gpsimd.dma_start(out=ones_t[:], in_=ones_ap)
            nc.gpsimd.dma_start(out=bt[64:65, :], in_=biases_ap[0:1, :])
            nc.vector.tensor_copy(bc_t[:], bc_raw[:])  # bf16 -> f32 scalars
            dma_w_half(wk_t, OFF_WK, 0, split=True)
            dma_w_half(wk_t, OFF_WK, 1)

            def qk_tile(dst_ap_fn, w_t, bj, xs, m):
                # one [128, HS] output tile of a q/k-style projection;
                # bias is folded into the PSUM->SBUF copy (per-partition add)
                ps = psA.tile([128, HS], f32, tag="pp")
                for kc in range(NE):
                    nc.tensor.matmul(ps[:], w_t[:, kc, m * 128:(m + 1) * 128],
                                     xs[:, kc, :], start=(kc == 0),
                                     stop=(kc == NE - 1))
                nc.vector.tensor_scalar(dst_ap_fn(), ps[:],
                                        bc_t[:, bj + m:bj + m + 1], None,
                                        op0=mybir.AluOpType.add)

            def v_tile(xs, sc, mm):
                # one [128 S-rows, 512 v-dims] tile of the V projection.
                # bv is folded into the out-proj bias on the host (softmax
                # rows sum to 1, so attn@(Vx+bv) == attn@Vx + bv): no bias
                # matmul here.
                ps = psA.tile([128, HC], f32, tag="pp")
                for kc in range(NE):
                    nc.tensor.matmul(ps[:], xs[:, kc, mm * 128:(mm + 1) * 128],
                                     wv_t[:, kc, :], start=(kc == 0),
                                     stop=(kc == NE - 1))
                nc.vector.tensor_copy(
                    v4[:, sc, :, 0:64],
                    ps[:].rearrange("p (h j) -> p h j", h=8))

            qtiles = [None] * NQT
            atps = [None] * NQT
            xv_cur = [None]
            xq_cur = [None]
            pp_done = [0] * NQT

            def proj_tasks(n):
                # v-slab n + q-slab n as resumable tile tasks
                qtiles[n] = qtp.tile([128, 4, SQ], bf16, tag="qt",
                                     name=f"qtile{n}")
                tasks = []
                for half in range(2):
                    for mm in range(2):
                        tasks.append(("v", n, mm, half, 0))
                    for m in range(4):
                        tasks.append(("q", n, m, half, 0))
                return tasks

            def partial_tile(part, j, mm):
                # my-rows contraction for output block j (rank j's columns),
                # s-rows mm; + bias/2 (DVE add of the pre-broadcast bias
                # tile -- no bias matmul). Summed across the pair by the
                # ReduceScatter that fires after the 8th tile.
                at_t = atps[part]
                wo_t = holder["wo_t"]
                po = psA.tile([128, HC], f32, tag="pp")
                for ic in range(4):
                    nc.tensor.matmul(po[:], at_t[:, ic, mm * 128:(mm + 1) * 128],
                                     wo_t[:, ic, j * HC:(j + 1) * HC],
                                     start=(ic == 0), stop=(ic == 3))
                okey = ("ob", part)
                if okey not in holder:
                    holder[okey] = opo.tile([128, 2, 4, HC], bf16, tag="ot",
                                            name=f"ob{part}")
                ob = holder[okey]
                nc.vector.tensor_tensor(ob[:, j, mm, :], po[:],
                                        bias_bc[:, j, :],
                                        op=mybir.AluOpType.add)
                pp_done[part] += 1
                if pp_done[part] in (4, 8):
                    # contiguous stores on Pool so the wait for the partials
                    # doesn't head-of-line block a load queue -- only the
                    # collective rides behind them anyway
                    h2 = pp_done[part] // 4 - 1
                    sl = slice(2 * h2, 2 * h2 + 2)
                    for j2 in range(2):
                        nc.gpsimd.dma_start(out=rsin[part][h2].ap()[j2],
                                            in_=ob[:, j2, sl, :])
                    if skip_cc:
                        nc.sync.dma_start(out=out.ap()[part][:, sl, :],
                                          in_=rsin[part][h2].ap()[0])
                    else:
                        # collectives cannot write IO tensors: scatter into
                        # scratch, then DRAM->DRAM DMA into the output
                        nc.gpsimd.collective_compute(
                            "ReduceScatter", mybir.AluOpType.add,
                            replica_groups=RG,
                            ins=[rsin[part][h2].ap().opt()],
                            outs=[rsout[part][h2].ap().opt()])
                        nc.sync.dma_start(out=out.ap()[part][:, sl, :],
                                          in_=rsout[part][h2].ap())

            pre = {}

            def run_task(t):
                kind, n, m, half = t[0], t[1], t[2], t[3]
                if kind == "v":
                    if m == 0:
                        xv_cur[0] = pre.pop(("v", n, half), None) or \
                            load_half("v", n, half, eng=nc.scalar)
                    v_tile(xv_cur[0], n * 4 + half * 2 + m, m)
                elif kind == "q":
                    if m == 0:
                        xq_cur[0] = pre.pop(("q", n, half), None) or \
                            load_half("q", n, half, eng=nc.scalar)
                    qtl = qtiles[n]
                    qk_tile(lambda: qtl[:, m, half * HS:(half + 1) * HS],
                            wq_t, 0, xq_cur[0], m)
                else:
                    partial_tile(n, m, half)

            # ---------------- prefix: full K projection ----------------
            # alternate slab loads across both HWDGE queues (scalar is idle
            # until the merged phase) so DMA arrival outruns PE consumption
            for n in range(4):
                for half in range(2):
                    first = n == 0 and half == 0
                    xs = load_half("k", n, half, split=first,
                                   eng=nc.scalar if (n * 2 + half) % 2 == 0
                                   else nc.sync)
                    if n == 0 and half == 1:
                        # masks/vones are not consumed until attention:
                        # keep them off the critical first-slab DMA window
                        nc.gpsimd.dma_start(out=masks_t[:], in_=masks_ap)
                        nc.gpsimd.memset(v4[:, :, :, 64], 1.0)
                    if n == 2 and half == 0:
                        # v/q weights land well before the merged phase
                        dma_w(wv_t, OFF_WV)
                        dma_w(wq_t, OFF_WQ)
                    if n == 3 and half == 1:
                        # prefetch the first merged-phase x slabs so the
                        # proj_tasks(0) stretch doesn't stall on the ACT
                        # queue's first transfer
                        pre[("v", 0, 0)] = load_half("v", 0, 0,
                                                     eng=nc.scalar)
                        pre[("q", 0, 0)] = load_half("q", 0, 0,
                                                     eng=nc.scalar)
                    for m in range(4):
                        off = n * SQ + half * HS
                        qk_tile(lambda m=m, off=off: kT[:, m, off:off + HS],
                                wk_t, 4, xs, m)
            # broadcast bo_half to all 128 partitions once (replaces the
            # per-partial-tile bias matmul)
            for j in range(2):
                pb0 = psA.tile([128, HC], f32, tag="pp")
                nc.tensor.matmul(pb0[:], ones_t[64:65, 0:128],
                                 bt[64:65, HC + j * HC:HC + (j + 1) * HC],
                                 start=True, stop=True)
                nc.vector.tensor_copy(bias_bc[:, j, :], pb0[:])

            # wk's SBUF slot is handed to wo_t; attention pools open here
            wkp_cm.__exit__(None, None, None)
            wop_cm = tc.tile_pool(name="wop", bufs=1)
            wop = wop_cm.__enter__()
            wo_t = wop.tile([128, 4, 2 * HC], bf16)
            holder["wo_t"] = wo_t
            holder["cm"] = wop_cm

            # ---------------- merged v/q projections + attention ----------
            for t in proj_tasks(0):
                run_task(t)
            nc.sync.dma_start(out=wo_t[:, 0:2, :], in_=wo_ap[:, 0:2, :])
            nc.sync.dma_start(out=wo_t[:, 2:4, :], in_=wo_ap[:, 2:4, :])

            work = []
            pending_fin = None
            for qt in range(NQT):
                if qt > 0:
                    # partial out-proj of the previous q-tile: local data
                    # only, safe to interleave from h0. mm-major so the
                    # first-half ReduceScatter fires after 4 tiles.
                    work.extend(("pp", qt - 1, j, mm, 0)
                                for mm in range(4) for j in range(2))
                if qt + 1 < NQT:
                    work.extend(proj_tasks(qt + 1))
                atps[qt] = atp.tile([128, 4, SQ], bf16, tag="at",
                                    name=f"atp{qt}")
                for h in range(8):
                    m, po = h // 2, 64 * (h % 2)
                    # rows 0-63: AV out; row 64: softmax denominator; rows
                    # 64-127 are reused by finalize's broadcast (after the
                    # reciprocal has consumed row 64) -- saves a PSUM bank
                    pav = psAV.tile([128, SQ], f32, tag="av")
                    nkc = (qt + 1) * (SQ // SK)
                    pts = {}
                    qtl = qtiles[qt]

                    def issue_score(kc, qt=qt, m=m, po=po, pts=pts, qtl=qtl):
                        r = kc - 4 * qt
                        # diagonal blocks: columns < r*128 are fully masked;
                        # restrict the score matmul too
                        s0 = r * SK if r in (1, 2, 3) else 0
                        pscore = psS.tile([128, SQ], f32, tag="sc")
                        nc.tensor.matmul(
                            pscore[:, s0:],
                            kT[po:po + 64, m, kc * SK:(kc + 1) * SK],
                            qtl[po:po + 64, m, s0:],
                            start=True, stop=True)
                        pt = att.tile([128, SQ], bf16, tag="pt")
                        if 0 <= r <= 3:
                            # columns < r*128 are fully masked: skip them
                            c0 = r * SK
                            praw = attr.tile([128, SQ], bf16, tag="praw")
                            nc.scalar.activation(praw[:, c0:], pscore[:, c0:],
                                                 Exp, scale=SCALE)
                            nc.vector.tensor_tensor(
                                pt[:, c0:], praw[:, c0:], masks_t[:, r, c0:],
                                op=mybir.AluOpType.mult)
                            pts[kc] = (pt, c0)
                        else:
                            nc.scalar.activation(pt[:], pscore[:], Exp,
                                                 scale=SCALE)
                            pts[kc] = (pt, 0)

                    for kc in range(min(lag, nkc)):
                        issue_score(kc)
                    # finalize the previous head while this head's score
                    # pipeline fills, so TensorE never waits on the DVE
                    # reciprocal chain
                    if pending_fin is not None:
                        pending_fin()
                        pending_fin = None
                    for kc in range(nkc):
                        if kc + lag < nkc:
                            issue_score(kc + lag)
                        pt, c0 = pts.pop(kc)
                        # kc==0 always has c0==0, so start covers the whole
                        # [65, 512] accumulator
                        nc.tensor.matmul(pav[0:65, c0:], v4[:, kc, h, :],
                                         pt[:, c0:],
                                         start=(kc == 0), stop=(kc == nkc - 1))

                    def finalize(qt=qt, h=h, pav=pav):
                        # normalized attnT rows land straight in the SBUF
                        # atp tile -- attnT never leaves the core
                        rt = attr.tile([1, SQ], bf16, tag="rt")
                        with nc.allow_low_precision(reason="bf16 pipeline"):
                            nc.vector.reciprocal(rt[:], pav[64:65, :])
                        nc.tensor.matmul(pav[64:128, :], ones_t[0:1, 0:64],
                                         rt[:], start=True, stop=True)
                        pbs = attr.tile([64, SQ], bf16, tag="pbs")
                        nc.vector.tensor_copy(pbs[:], pav[64:128, :])
                        p0 = 64 * (h % 2)
                        nc.vector.tensor_tensor(
                            atps[qt][p0:p0 + 64, h // 2, :],
                            pav[0:64, :], pbs[:], op=mybir.AluOpType.mult)

                    pending_fin = finalize
                    ready = sum(1 for t in work if t[4] <= h)
                    npop = -(-ready // max(1, eager - h)) if ready else 0
                    for _ in range(npop):
                        run_task(work.pop(0))
                if pending_fin is not None:
                    pending_fin()
                    pending_fin = None
                while work:
                    run_task(work.pop(0))
            # last q-tile's partial out-proj + its ReduceScatter
            for mm in range(4):
                for j in range(2):
                    partial_tile(3, j, mm)
            holder["cm"].__exit__(None, None, None)

    nc.compile()
    return nc


_NC_CACHE = None


def _get_nc():
    global _NC_CACHE
    if _NC_CACHE is None:
        _NC_CACHE = build_nc()
    return _NC_CACHE


def _prep_x(xT):
    # [E, S] -> slab-blocked [n, half, p, c, s'] (contiguous per partition)
    a = xT.reshape(NE, 128, NQT, 2, HS)          # (c, p, n, half, s')
    return np.ascontiguousarray(a.transpose(2, 3, 1, 0, 4))


def _prep_w(wT):
    # [E, HC] -> half-blocked [half, p, c', n]
    a = wT.reshape(2, NE // 2, 128, HC)          # (half, c', p, n)
    return np.ascontiguousarray(a.transpose(0, 2, 1, 3))


def _prepare_in_maps(query, key, value, Wq, bq, Wk, bk, Wv, bv, Wo, bo):
    import ml_dtypes
    bfl = ml_dtypes.bfloat16

    query = np.asarray(query, dtype=np.float32)
    key = np.asarray(key, dtype=np.float32)
    value = np.asarray(value, dtype=np.float32)

    xT = [[_prep_x(np.ascontiguousarray(a[b].T).astype(bfl)) for b in range(B)]
          for a in (query, key, value)]

    p = np.arange(128)[:, None, None]
    r = np.arange(4)[None, :, None]
    qn = np.arange(SQ)[None, None, :]
    masks = ((p + r * 128) <= qn).astype(bfl)
    ones = np.ones((65, SQ), dtype=bfl)

    WoT = np.ascontiguousarray(np.asarray(Wo).T)   # [E, 1024] all columns
    # softmax rows sum to 1, so attn@(Vx+bv) == attn@Vx + bv: fold bv into
    # the out-proj bias (exact)
    bo_fold = (np.asarray(bv, dtype=np.float64) @ np.asarray(WoT, np.float64)
               + np.asarray(bo, np.float64)).astype(np.float32)
    bo_half = (bo_fold * 0.5).astype(np.float32)

    w_g, biases_g, bcol_g, wo_g = [], [], [], []
    for g in range(2):
        sl = slice(g * HC, (g + 1) * HC)
        ws = [_prep_w(np.ascontiguousarray(np.asarray(W)[sl, :].T).astype(bfl))
              for W in (Wq, Wk, Wv)]
        w_g.append(ws)
        # out-proj: MY 512 rows (= my heads' dims) x ALL 1024 columns,
        # [p, ic, n] blocked
        wo_my = WoT[g * HC:(g + 1) * HC, :].reshape(4, 128, 2 * HC)
        wo_g.append(np.ascontiguousarray(
            wo_my.transpose(1, 0, 2)).astype(bfl))
        biases_g.append(np.concatenate([
            np.zeros(HC, np.float32), bo_half,
            np.zeros(HC, np.float32)]).reshape(1, 4 * HC).astype(bfl))
        bc = np.empty((128, 8), dtype=np.float32)
        for mj in range(4):
            bc[:, mj] = np.asarray(bq)[g * HC + mj * 128:g * HC + (mj + 1) * 128]
            bc[:, 4 + mj] = np.asarray(bk)[g * HC + mj * 128:g * HC + (mj + 1) * 128]
        bcol_g.append(bc)

    in_maps = []
    blob_cache = {}
    for c in range(N_CORES):
        b, g = c // 2, c % 2
        key_ = (b, g)
        if key_ not in blob_cache:
            parts = [xT[0][b], xT[1][b], xT[2][b]] + w_g[g] + \
                    [wo_g[g], biases_g[g], masks, ones,
                     bcol_g[g].astype(bfl)]
            blob = np.concatenate([np.ascontiguousarray(a).reshape(-1)
                                   for a in parts])
            assert blob.shape[0] == BLOB_LEN, (blob.shape, BLOB_LEN)
            blob_cache[key_] = blob
        in_maps.append({"blob": blob_cache[key_]})
    return in_maps


def run(trace=False, **inputs):
    in_maps = _prepare_in_maps(**inputs)
    nc = _get_nc()
    res = bass_utils.run_bass_kernel_spmd(
        nc, in_maps, core_ids=list(range(N_CORES)), trace=trace)
    full = np.empty((B, S, E), dtype=np.float32)
    for c in range(N_CORES):
        b, g = c // 2, c % 2
        # out rows are [qt, p, mm] blocked: s = qt*SQ + mm*128 + p
        o = res.results[c]["out"].astype(np.float32)
        full[b, :, g * HC:(g + 1) * HC] = o.transpose(0, 2, 1, 3).reshape(
            S, HC)
    return full, res


def kernel(**inputs) -> np.ndarray:
    full, _ = run(trace=False, **inputs)
    return full


def bench(n_iters=5, repeats=5, nc=None, **inputs):
    """Estimate on-device NEFF time: chain n_iters executions with a tiny
    data dependency (no CSE, strict serialization), time with device-resident
    inputs, and report the marginal per-iteration wall time."""
    import time
    import jax
    from jax.sharding import Mesh, PartitionSpec
    from jax.experimental.shard_map import shard_map
    import concourse.bass2jax as bass2jax
    import concourse.mybir as mb

    if nc is None:
        nc = _get_nc()
    in_maps = _prepare_in_maps(**inputs)
    bass2jax.install_neuronx_cc_hook()

    partition_name = nc.partition_id_tensor.name if nc.partition_id_tensor else None
    in_names, out_names, out_avals = [], [], []
    for alloc in nc.m.functions[0].allocations:
        if not isinstance(alloc, mb.MemoryLocationSet):
            continue
        name = alloc.memorylocations[0].name
        if alloc.kind == "ExternalInput":
            if name != partition_name:
                in_names.append(name)
        elif alloc.kind == "ExternalOutput":
            out_names.append(name)
            out_avals.append(
                jax.core.ShapedArray(tuple(alloc.tensor_shape),
                                     mb.dt.np(alloc.dtype)))
    n_params = len(in_names)
    all_in_names = list(in_names) + list(out_names)
    if partition_name is not None:
        all_in_names.append(partition_name)

    def _body(*args):
        operands = list(args)
        if partition_name is not None:
            operands.append(bass2jax.partition_id_tensor())
        outs = bass2jax._bass_exec_p.bind(
            *operands,
            out_avals=tuple(out_avals),
            in_names=tuple(all_in_names),
            out_names=tuple(out_names),
            lowering_input_output_aliases=(),
            sim_require_finite=True,
            sim_require_nnan=True,
            nc=nc)
        return tuple(outs)

    devices = jax.devices()[:N_CORES]
    mesh = Mesh(np.asarray(devices), ("core",))
    n_outs = len(out_names)
    in_specs = (PartitionSpec("core"),) * (n_params + n_outs)
    out_specs = (PartitionSpec("core"),) * n_outs

    per_core = [[np.asarray(m[name]) for name in in_names] for m in in_maps]
    concat_in = [np.concatenate([per_core[c][i] for c in range(N_CORES)], axis=0)
                 for i in range(n_params)]
    concat_zeros = [np.zeros((N_CORES * a.shape[0], *a.shape[1:]), a.dtype)
                    for a in out_avals]

    sharding = jax.sharding.NamedSharding(mesh, PartitionSpec("core"))
    dev_in = [jax.device_put(x, sharding) for x in concat_in + concat_zeros]

    # donate the output buffers and chain each call's outputs into the next
    # call's donated outputs: executions serialize on-device, memory stays
    # bounded, and M iterations aggregate enough device time to dominate the
    # ~100ms axon RTT quantum.
    donate = tuple(range(n_params, n_params + n_outs))
    fn = jax.jit(shard_map(_body, mesh=mesh, in_specs=in_specs,
                           out_specs=out_specs, check_rep=False),
                 keep_unused=True, donate_argnums=donate)
    params = dev_in[:n_params]
    outs = tuple(dev_in[n_params:])
    outs = fn(*params, *outs)  # warm
    jax.block_until_ready(outs)

    def run_m(m):
        nonlocal outs
        t0 = time.perf_counter()
        for _ in range(m):
            outs = fn(*params, *outs)
        jax.block_until_ready(outs)
        return time.perf_counter() - t0

    # The axon dispatch path is bimodal (~48ms vs ~84ms floor for a short
    # chain), so a two-point min-min marginal can be wildly off when the two
    # points land in different states. Use the median at each of three chain
    # lengths and a least-squares slope instead.
    ms = [8, 8 + n_iters // 2, 8 + n_iters]
    med = {}
    for m in ms:
        ts = sorted(run_m(m) for _ in range(repeats))
        med[m] = ts[len(ts) // 2]
    mean_m = sum(ms) / len(ms)
    mean_t = sum(med.values()) / len(ms)
    slope = (sum((m - mean_m) * (med[m] - mean_t) for m in ms)
             / sum((m - mean_m) ** 2 for m in ms))
    return slope * 1e9, {m: med[m] for m in ms}



# revision 50
# speedup vs baseline: 1.9564x; 1.1258x over previous
"""Distributed causal multi-head attention block for Trainium2 (8 NeuronCores).

Problem: B=4, S=2048, E=1024, H=16 heads, fp32.
    q/k/v = Linear(query/key/value); causal softmax attention; out = Linear(attn).

Sharding: DP=4 over batch x TP=2 over heads. Core c = 2*b + g handles batch b
with heads [8g, 8g+8). Per-core kernel structure (single fused Tile graph):
  - K projection prefix (kT in d-major layout), then a software-pipelined
    merged phase: per q-tile, the next q-tile's V/Q projection tiles and the
    previous q-tile's partial out-projections are interleaved between
    attention heads so TensorE fills the ACT(exp)-bound stretches.
  - Attention is computed in the *transposed* orientation, scoresT[k, q]:
    no max-subtraction (scores are O(1) by construction), no transposes
    anywhere; the softmax denominator comes from an extra ones-column in the
    AV matmul; normalization is a DVE reciprocal + rank-1 broadcast matmul
    whose result lands directly in an SBUF attnT tile (never leaves core).
  - Out-projection via partial sums + ReduceScatter: each core contracts its
    OWN 512 attnT dims against Wo.T[my rows, ALL 1024 columns] (host supplies
    this slice); bias/2 is folded in on the PSUM->SBUF copy via a
    pre-broadcast bias tile (no per-tile bias matmuls). Partials collect in
    one staging tile per q-tile and ship as two contiguous stores; TWO
    pairwise ReduceScatter(add)s per q-tile (s-halves) fire as soon as their
    4 partials exist, so the last RS overlaps the last partials. Scatter goes
    to scratch (collectives cannot touch IO tensors), then a contiguous
    D2D copy lands each core's 512 output columns in `out`, which is
    [qt, p, mm, e]-blocked (host unpermutes). No attnT exchange, no
    gather-dependent matmuls, fully rank-symmetric (SPMD-safe).

DMA discipline: all big inputs ride in ONE bf16 blob laid out so every load
is contiguous per SBUF partition (minimal descriptors); prefix xk slabs
alternate across the SP and Activation HWDGE queues (first slab + first wk
half split per-chunk so the first matmul starts ~3us earlier); merged-phase
xq/xv loads ride the Activation queue (first slab prefetched during the
prefix); partial stores + collectives on the Pool queue. No engine queue
head-of-line blocks another engine's critical work.

The softmax-normalize broadcast shares the AV accumulator's PSUM bank
(rows 64-127, written after the reciprocal consumes the denominator row),
freeing a bank so the score pipeline runs at lag=3.

All matmul accumulation is fp32 in PSUM; error vs the fp32 reference is
bf16 input rounding (~5e-3 relative).
"""
import sys

if "/opt/trn_rl_repo" not in sys.path:
    sys.path.insert(0, "/opt/trn_rl_repo")

import numpy as np

import concourse.bacc as bacc
import concourse.tile as tile
import concourse.mybir as mybir
import concourse.bass_utils as bass_utils

f32 = mybir.dt.float32
bf16 = mybir.dt.bfloat16
Exp = mybir.ActivationFunctionType.Exp

N_CORES = 8
B, S, E = 4, 2048, 1024
H, D = 16, 64
HC = 512            # per-core head dims (8 heads x 64)
SCALE = D ** -0.5
SQ = 512            # q-tile width (columns of scoresT)
SK = 128            # k-chunk (partition rows of scoresT)
NQT = S // SQ       # 4 q-tiles
NE = E // 128       # 8 contraction chunks of the E dim
HS = SQ // 2        # 256-wide half-slabs of the input stream

# blob element offsets (bf16 elements). x streams are stored slab-blocked
# [n, half, p, c, s'], q/k/v weights half-blocked [half, p, c', n], and the
# out-proj weight [p, ic, n_all] so every DMA is contiguous per partition.
_XSZ = E * S
_WSZ = E * HC
_SLAB = 128 * NE * HS          # one x half-slab
_WHALF = 128 * (NE // 2) * HC  # one q/k/v weight half
OFF_XQ = 0
OFF_XK = OFF_XQ + _XSZ
OFF_XV = OFF_XK + _XSZ
OFF_WQ = OFF_XV + _XSZ
OFF_WK = OFF_WQ + _WSZ
OFF_WV = OFF_WK + _WSZ
OFF_WO = OFF_WV + _WSZ                # [p, ic, 2*HC] = my rows x all cols
OFF_BIASES = OFF_WO + 128 * 4 * 2 * HC  # [1, 2048]: bv | bo_all/2 | pad
OFF_MASKS = OFF_BIASES + 4 * HC       # [128, 4, SQ]
OFF_ONES = OFF_MASKS + 128 * 4 * SQ   # [65, SQ]
OFF_BCOL = OFF_ONES + 65 * SQ         # [128, 8]: bq[m] (j=m) / bk[m] (4+m)
BLOB_LEN = OFF_BCOL + 128 * 8


def build_nc(skip_cc=False, lag=3, eager=10, pad_elems=0):
    nc = bacc.Bacc("TRN2", target_bir_lowering=False, debug=False,
                   num_devices=N_CORES)

    blob = nc.declare_dram_parameter("blob", [BLOB_LEN], bf16, isOutput=False)
    if pad_elems:
        nc.declare_dram_parameter("pad", [pad_elems], bf16, isOutput=False)
    # out rows are [qt, p, mm] blocked (s = qt*SQ + mm*128 + p); the host
    # unpermutes. This keeps every rsin/rsout/out DMA fully contiguous.
    out = nc.declare_dram_parameter("out", [NQT, 128, 4, HC], bf16,
                                    isOutput=True)

    def bview(off, pattern, **sizes):
        import re
        names = re.findall(r"\w+", pattern.split("->")[0])
        length = 1
        for n in names:
            length *= sizes[n]
        return blob.ap()[off:off + length].rearrange(
            f"({' '.join(names)}) -> {pattern.split('->')[1].strip()}", **sizes)

    def slab_view(stream, n, half):
        base = AGOFF[stream]
        return bview(base + (n * 2 + half) * _SLAB,
                     "p c s -> p c s", p=128, c=NE, s=HS)

    def whalf_view(base, half):
        return bview(base + half * _WHALF,
                     "p c n -> p c n", p=128, c=NE // 2, n=HC)

    wo_ap = bview(OFF_WO, "p c n -> p c n", p=128, c=4, n=2 * HC)
    biases_ap = bview(OFF_BIASES, "r n -> r n", r=1, n=4 * HC)
    masks_ap = bview(OFF_MASKS, "p r q -> p r q", p=128, r=4, q=SQ)
    ones_ap = bview(OFF_ONES, "a q -> a q", a=65, q=SQ)
    bcol_ap = bview(OFF_BCOL, "p j -> p j", p=128, j=8)

    # ReduceScatter staging: per q-tile, my partial out-proj for BOTH cores'
    # output columns, j-major [j, p, mm, e] so block j is one contiguous
    # half; the collective sums pairs and scatters block g to core g.
    # Two collectives per q-tile (mm 0-1 / mm 2-3) so the first fires while
    # the second half's partials are still on TensorE.
    rsin = [[nc.dram_tensor(f"rsin{i}_{h}", [2, 128, 2, HC], bf16)
             for h in range(2)] for i in range(4)]
    rsout = [[nc.dram_tensor(f"rsout{i}_{h}", [128, 2, HC], bf16)
              for h in range(2)] for i in range(4)]
    RG = [[0, 1], [2, 3], [4, 5], [6, 7]]

    AGOFF = {"q": OFF_XQ, "k": OFF_XK, "v": OFF_XV}

    with tile.TileContext(nc) as tc:
        with tc.tile_pool(name="persist", bufs=1) as pp, \
             tc.tile_pool(name="xsp", bufs=4) as xsp, \
             tc.tile_pool(name="qtp", bufs=2) as qtp, \
             tc.tile_pool(name="atp", bufs=3) as atp, \
             tc.tile_pool(name="att", bufs=lag + 3) as att, \
             tc.tile_pool(name="attr", bufs=5) as attr, \
             tc.tile_pool(name="opo", bufs=2) as opo, \
             tc.tile_pool(name="psA", bufs=2, space="PSUM") as psA, \
             tc.tile_pool(name="psS", bufs=2, space="PSUM") as psS, \
             tc.tile_pool(name="psAV", bufs=2, space="PSUM") as psAV:
            kT = pp.tile([128, 4, S], bf16)       # [p, m, s]: k-dim = m*128+p
            v4 = pp.tile([128, 16, 8, 65], bf16)  # [p, sc, h, j]: v row sc*128+p
            masks_t = pp.tile([128, 4, SQ], bf16)
            ones_t = pp.tile([65, SQ], bf16)
            bt = pp.tile([65, 4 * HC], bf16)  # p64: bv | bo_all/2 | pad
            bc_raw = pp.tile([128, 8], bf16)
            bc_t = pp.tile([128, 8], f32)     # col j: bq[m] (j=m) / bk[m] (4+m)
            bias_bc = pp.tile([128, 2, HC], bf16)  # bo_half bcast to 128 rows
            wq_t = pp.tile([128, NE, HC], bf16)
            wv_t = pp.tile([128, NE, HC], bf16)
            # wk lives in its own pool: its slot is handed to wo mid-loop,
            # after the last kT tile is produced
            wkp_cm = tc.tile_pool(name="wkp", bufs=1)
            wkp = wkp_cm.__enter__()
            wk_t = wkp.tile([128, NE, HC], bf16)
            holder = {}

            def dma_w_half(dst, base, i, split=False):
                half = NE // 2
                src_ap = whalf_view(base, i)
                if split:
                    for kc in range(half):
                        nc.sync.dma_start(out=dst[:, i * half + kc, :],
                                          in_=src_ap[:, kc, :])
                else:
                    nc.sync.dma_start(
                        out=dst[:, i * half:(i + 1) * half, :], in_=src_ap)

            def dma_w(dst, base):
                dma_w_half(dst, base, 0)
                dma_w_half(dst, base, 1)

            def load_half(stream, n, half, split=False, eng=None):
                eng = eng or nc.sync
                xs = xsp.tile([128, NE, HS], bf16, tag="x")
                src_ap = slab_view(stream, n, half)
                if split:
                    for kc in range(NE):
                        eng.dma_start(out=xs[:, kc, :], in_=src_ap[:, kc, :])
                else:
                    eng.dma_start(out=xs[:], in_=src_ap)
                return xs

            # both wk halves first (the first qk_tile accumulates over all 8
            # chunks); the first half per-chunk so matmul kc=0 starts as soon
            # as 128KB (not 512KB) has landed. Small tiles ride gpsimd.
            nc.gpsimd.dma_start(out=bc_raw[:], in_=bcol_ap)
            nc.gpsimd.dma_start(out=ones_t[:], in_=ones_ap)
            nc.gpsimd.dma_start(out=bt[64:65, :], in_=biases_ap[0:1, :])
            nc.vector.tensor_copy(bc_t[:], bc_raw[:])  # bf16 -> f32 scalars
            dma_w_half(wk_t, OFF_WK, 0, split=True)
            dma_w_half(wk_t, OFF_WK, 1)

            def qk_tile(dst_ap_fn, w_t, bj, xs, m):
                # one [128, HS] output tile of a q/k-style projection;
                # bias is folded into the PSUM->SBUF copy (per-partition add)
                ps = psA.tile([128, HS], f32, tag="pp")
                for kc in range(NE):
                    nc.tensor.matmul(ps[:], w_t[:, kc, m * 128:(m + 1) * 128],
                                     xs[:, kc, :], start=(kc == 0),
                                     stop=(kc == NE - 1))
                nc.vector.tensor_scalar(dst_ap_fn(), ps[:],
                                        bc_t[:, bj + m:bj + m + 1], None,
                                        op0=mybir.AluOpType.add)

            def v_tile(xs, sc, mm):
                # one [128 S-rows, 512 v-dims] tile of the V projection.
                # bv is folded into the out-proj bias on the host (softmax
                # rows sum to 1, so attn@(Vx+bv) == attn@Vx + bv): no bias
                # matmul here.
                ps = psA.tile([128, HC], f32, tag="pp")
                for kc in range(NE):
                    nc.tensor.matmul(ps[:], xs[:, kc, mm * 128:(mm + 1) * 128],
                                     wv_t[:, kc, :], start=(kc == 0),
                                     stop=(kc == NE - 1))
                nc.vector.tensor_copy(
                    v4[:, sc, :, 0:64],
                    ps[:].rearrange("p (h j) -> p h j", h=8))

            qtiles = [None] * NQT
            atps = [None] * NQT
            xv_cur = [None]
            xq_cur = [None]
            pp_done = [0] * NQT

            def proj_tasks(n):
                # v-slab n + q-slab n as resumable tile tasks
                qtiles[n] = qtp.tile([128, 4, SQ], bf16, tag="qt",
                                     name=f"qtile{n}")
                tasks = []
                for half in range(2):
                    for mm in range(2):
                        tasks.append(("v", n, mm, half, 0))
                    for m in range(4):
                        tasks.append(("q", n, m, half, 0))
                return tasks

            def partial_tile(part, j, mm):
                # my-rows contraction for output block j (rank j's columns),
                # s-rows mm; + bias/2 (DVE add of the pre-broadcast bias
                # tile -- no bias matmul). Summed across the pair by the
                # ReduceScatter that fires after the 8th tile.
                at_t = atps[part]
                wo_t = holder["wo_t"]
                po = psA.tile([128, HC], f32, tag="pp")
                for ic in range(4):
                    nc.tensor.matmul(po[:], at_t[:, ic, mm * 128:(mm + 1) * 128],
                                     wo_t[:, ic, j * HC:(j + 1) * HC],
                                     start=(ic == 0), stop=(ic == 3))
                okey = ("ob", part)
                if okey not in holder:
                    holder[okey] = opo.tile([128, 2, 4, HC], bf16, tag="ot",
                                            name=f"ob{part}")
                ob = holder[okey]
                nc.vector.tensor_tensor(ob[:, j, mm, :], po[:],
                                        bias_bc[:, j, :],
                                        op=mybir.AluOpType.add)
                pp_done[part] += 1
                if pp_done[part] in (4, 8):
                    # contiguous stores on Pool so the wait for the partials
                    # doesn't head-of-line block a load queue -- only the
                    # collective rides behind them anyway
                    h2 = pp_done[part] // 4 - 1
                    sl = slice(2 * h2, 2 * h2 + 2)
                    for j2 in range(2):
                        nc.gpsimd.dma_start(out=rsin[part][h2].ap()[j2],
                                            in_=ob[:, j2, sl, :])
                    if skip_cc:
                        nc.sync.dma_start(out=out.ap()[part][:, sl, :],
                                          in_=rsin[part][h2].ap()[0])
                    else:
                        # collectives cannot write IO tensors: scatter into
                        # scratch, then DRAM->DRAM DMA into the output
                        nc.gpsimd.collective_compute(
                            "ReduceScatter", mybir.AluOpType.add,
                            replica_groups=RG,
                            ins=[rsin[part][h2].ap().opt()],
                            outs=[rsout[part][h2].ap().opt()])
                        nc.sync.dma_start(out=out.ap()[part][:, sl, :],
                                          in_=rsout[part][h2].ap())

            pre = {}

            def run_task(t):
                kind, n, m, half = t[0], t[1], t[2], t[3]
                if kind == "v":
                    if m == 0:
                        xv_cur[0] = pre.pop(("v", n, half), None) or \
                            load_half("v", n, half, eng=nc.scalar)
                    v_tile(xv_cur[0], n * 4 + half * 2 + m, m)
                elif kind == "q":
                    if m == 0:
                        xq_cur[0] = pre.pop(("q", n, half), None) or \
                            load_half("q", n, half, eng=nc.scalar)
                    qtl = qtiles[n]
                    qk_tile(lambda: qtl[:, m, half * HS:(half + 1) * HS],
                            wq_t, 0, xq_cur[0], m)
                else:
                    partial_tile(n, m, half)

            # ---------------- prefix: full K projection ----------------
            # alternate slab loads across both HWDGE queues (scalar is idle
            # until the merged phase) so DMA arrival outruns PE consumption
            for n in range(4):
                for half in range(2):
                    first = n == 0 and half == 0
                    xs = load_half("k", n, half, split=first,
                                   eng=nc.scalar if (n * 2 + half) % 2 == 0
                                   else nc.sync)
                    if n == 0 and half == 1:
                        # masks/vones are not consumed until attention:
                        # keep them off the critical first-slab DMA window
                        nc.gpsimd.dma_start(out=masks_t[:], in_=masks_ap)
                        nc.gpsimd.memset(v4[:, :, :, 64], 1.0)
                    if n == 2 and half == 0:
                        # v/q weights land well before the merged phase
                        dma_w(wv_t, OFF_WV)
                        dma_w(wq_t, OFF_WQ)
                    if n == 3 and half == 1:
                        # prefetch the first merged-phase x slabs so the
                        # proj_tasks(0) stretch doesn't stall on the ACT
                        # queue's first transfer
                        pre[("v", 0, 0)] = load_half("v", 0, 0,
                                                     eng=nc.scalar)
                        pre[("q", 0, 0)] = load_half("q", 0, 0,
                                                     eng=nc.scalar)
                    for m in range(4):
                        off = n * SQ + half * HS
                        qk_tile(lambda m=m, off=off: kT[:, m, off:off + HS],
                                wk_t, 4, xs, m)
            # broadcast bo_half to all 128 partitions once (replaces the
            # per-partial-tile bias matmul)
            for j in range(2):
                pb0 = psA.tile([128, HC], f32, tag="pp")
                nc.tensor.matmul(pb0[:], ones_t[64:65, 0:128],
                                 bt[64:65, HC + j * HC:HC + (j + 1) * HC],
                                 start=True, stop=True)
                nc.vector.tensor_copy(bias_bc[:, j, :], pb0[:])

            # wk's SBUF slot is handed to wo_t; attention pools open here
            wkp_cm.__exit__(None, None, None)
            wop_cm = tc.tile_pool(name="wop", bufs=1)
            wop = wop_cm.__enter__()
            wo_t = wop.tile([128, 4, 2 * HC], bf16)
            holder["wo_t"] = wo_t
            holder["cm"] = wop_cm

            # ---------------- merged v/q projections + attention ----------
            for t in proj_tasks(0):
                run_task(t)
            nc.sync.dma_start(out=wo_t[:, 0:2, :], in_=wo_ap[:, 0:2, :])
            nc.sync.dma_start(out=wo_t[:, 2:4, :], in_=wo_ap[:, 2:4, :])

            work = []
            pending_fin = None
            for qt in range(NQT):
                if qt > 0:
                    # partial out-proj of the previous q-tile: local data
                    # only, safe to interleave from h0. mm-major so the
                    # first-half ReduceScatter fires after 4 tiles.
                    work.extend(("pp", qt - 1, j, mm, 0)
                                for mm in range(4) for j in range(2))
                if qt + 1 < NQT:
                    work.extend(proj_tasks(qt + 1))
                atps[qt] = atp.tile([128, 4, SQ], bf16, tag="at",
                                    name=f"atp{qt}")
                for h in range(8):
                    m, po = h // 2, 64 * (h % 2)
                    # rows 0-63: AV out; row 64: softmax denominator; rows
                    # 64-127 are reused by finalize's broadcast (after the
                    # reciprocal has consumed row 64) -- saves a PSUM bank
                    pav = psAV.tile([128, SQ], f32, tag="av")
                    nkc = (qt + 1) * (SQ // SK)
                    pts = {}
                    qtl = qtiles[qt]

                    def issue_pair(pi, qt=qt, m=m, po=po, pts=pts, qtl=qtl):
                        # two k-chunks share one 2-bank PSUM tile, ONE exp,
                        # and (diagonal) ONE combined mask multiply: halves
                        # the ACT/DVE instruction count on the attention
                        # chain. Masked columns multiply to 0; exp of the
                        # never-written gap columns is junk that no AV
                        # matmul ever reads.
                        pscore = psS.tile([128, 2, SQ], f32, tag="sc")
                        c0s = []
                        for t_ in range(2):
                            kc = 2 * pi + t_
                            r = kc - 4 * qt
                            s0 = r * SK if r in (1, 2, 3) else 0
                            c0s.append(s0)
                            nc.tensor.matmul(
                                pscore[:, t_, s0:],
                                kT[po:po + 64, m, kc * SK:(kc + 1) * SK],
                                qtl[po:po + 64, m, s0:],
                                start=True, stop=True)
                        pt = att.tile([128, 2, SQ], bf16, tag="pt")
                        ps_flat = pscore[:].rearrange("p a q -> p (a q)")
                        pt_flat = pt[:].rearrange("p a q -> p (a q)")
                        c0 = c0s[0]
                        rA = 2 * pi - 4 * qt
                        if 0 <= rA <= 3:
                            praw = attr.tile([128, 2, SQ], bf16, tag="praw")
                            praw_flat = praw[:].rearrange("p a q -> p (a q)")
                            mk = masks_t[:, rA:rA + 2, :].rearrange(
                                "p r q -> p (r q)")
                            nc.scalar.activation(praw_flat[:, c0:],
                                                 ps_flat[:, c0:],
                                                 Exp, scale=SCALE)
                            nc.vector.tensor_tensor(
                                pt_flat[:, c0:], praw_flat[:, c0:],
                                mk[:, c0:], op=mybir.AluOpType.mult)
                        else:
                            nc.scalar.activation(pt_flat[:], ps_flat[:],
                                                 Exp, scale=SCALE)
                        for t_ in range(2):
                            pts[2 * pi + t_] = (pt[:, t_, :], c0s[t_])

                    npairs = nkc // 2
                    for pi in range(min(2, npairs)):
                        issue_pair(pi)
                    # finalize the previous head while this head's score
                    # pipeline fills, so TensorE never waits on the DVE
                    # reciprocal chain
                    if pending_fin is not None:
                        pending_fin()
                        pending_fin = None
                    for kc in range(nkc):
                        nxt = kc + 4
                        if nxt < nkc and nxt % 2 == 0:
                            issue_pair(nxt // 2)
                        pt, c0 = pts.pop(kc)
                        # kc==0 always has c0==0, so start covers the whole
                        # [65, 512] accumulator
                        nc.tensor.matmul(pav[0:65, c0:], v4[:, kc, h, :],
                                         pt[:, c0:],
                                         start=(kc == 0), stop=(kc == nkc - 1))

                    def finalize(qt=qt, h=h, pav=pav):
                        # normalized attnT rows land straight in the SBUF
                        # atp tile -- attnT never leaves the core
                        rt = attr.tile([1, SQ], bf16, tag="rt")
                        with nc.allow_low_precision(reason="bf16 pipeline"):
                            nc.vector.reciprocal(rt[:], pav[64:65, :])
                        nc.tensor.matmul(pav[64:128, :], ones_t[0:1, 0:64],
                                         rt[:], start=True, stop=True)
                        pbs = attr.tile([64, SQ], bf16, tag="pbs")
                        nc.vector.tensor_copy(pbs[:], pav[64:128, :])
                        p0 = 64 * (h % 2)
                        nc.vector.tensor_tensor(
                            atps[qt][p0:p0 + 64, h // 2, :],
                            pav[0:64, :], pbs[:], op=mybir.AluOpType.mult)

                    pending_fin = finalize
                    ready = sum(1 for t in work if t[4] <= h)
                    npop = -(-ready // max(1, eager - h)) if ready else 0
                    for _ in range(npop):
                        run_task(work.pop(0))
                if pending_fin is not None:
                    pending_fin()
                    pending_fin = None
                while work:
                    run_task(work.pop(0))
            # last q-tile's partial out-proj + its ReduceScatter
            for mm in range(4):
                for j in range(2):
                    partial_tile(3, j, mm)
            holder["cm"].__exit__(None, None, None)

    nc.compile()
    return nc


_NC_CACHE = None


def _get_nc():
    global _NC_CACHE
    if _NC_CACHE is None:
        _NC_CACHE = build_nc()
    return _NC_CACHE


def _prep_x(xT):
    # [E, S] -> slab-blocked [n, half, p, c, s'] (contiguous per partition)
    a = xT.reshape(NE, 128, NQT, 2, HS)          # (c, p, n, half, s')
    return np.ascontiguousarray(a.transpose(2, 3, 1, 0, 4))


def _prep_w(wT):
    # [E, HC] -> half-blocked [half, p, c', n]
    a = wT.reshape(2, NE // 2, 128, HC)          # (half, c', p, n)
    return np.ascontiguousarray(a.transpose(0, 2, 1, 3))


def _prepare_in_maps(query, key, value, Wq, bq, Wk, bk, Wv, bv, Wo, bo):
    import ml_dtypes
    bfl = ml_dtypes.bfloat16

    query = np.asarray(query, dtype=np.float32)
    key = np.asarray(key, dtype=np.float32)
    value = np.asarray(value, dtype=np.float32)

    xT = [[_prep_x(np.ascontiguousarray(a[b].T).astype(bfl)) for b in range(B)]
          for a in (query, key, value)]

    p = np.arange(128)[:, None, None]
    r = np.arange(4)[None, :, None]
    qn = np.arange(SQ)[None, None, :]
    masks = ((p + r * 128) <= qn).astype(bfl)
    ones = np.ones((65, SQ), dtype=bfl)

    WoT = np.ascontiguousarray(np.asarray(Wo).T)   # [E, 1024] all columns
    # softmax rows sum to 1, so attn@(Vx+bv) == attn@Vx + bv: fold bv into
    # the out-proj bias (exact)
    bo_fold = (np.asarray(bv, dtype=np.float64) @ np.asarray(WoT, np.float64)
               + np.asarray(bo, np.float64)).astype(np.float32)
    bo_half = (bo_fold * 0.5).astype(np.float32)

    w_g, biases_g, bcol_g, wo_g = [], [], [], []
    for g in range(2):
        sl = slice(g * HC, (g + 1) * HC)
        ws = [_prep_w(np.ascontiguousarray(np.asarray(W)[sl, :].T).astype(bfl))
              for W in (Wq, Wk, Wv)]
        w_g.append(ws)
        # out-proj: MY 512 rows (= my heads' dims) x ALL 1024 columns,
        # [p, ic, n] blocked
        wo_my = WoT[g * HC:(g + 1) * HC, :].reshape(4, 128, 2 * HC)
        wo_g.append(np.ascontiguousarray(
            wo_my.transpose(1, 0, 2)).astype(bfl))
        biases_g.append(np.concatenate([
            np.zeros(HC, np.float32), bo_half,
            np.zeros(HC, np.float32)]).reshape(1, 4 * HC).astype(bfl))
        bc = np.empty((128, 8), dtype=np.float32)
        for mj in range(4):
            bc[:, mj] = np.asarray(bq)[g * HC + mj * 128:g * HC + (mj + 1) * 128]
            bc[:, 4 + mj] = np.asarray(bk)[g * HC + mj * 128:g * HC + (mj + 1) * 128]
        bcol_g.append(bc)

    in_maps = []
    blob_cache = {}
    for c in range(N_CORES):
        b, g = c // 2, c % 2
        key_ = (b, g)
        if key_ not in blob_cache:
            parts = [xT[0][b], xT[1][b], xT[2][b]] + w_g[g] + \
                    [wo_g[g], biases_g[g], masks, ones,
                     bcol_g[g].astype(bfl)]
            blob = np.concatenate([np.ascontiguousarray(a).reshape(-1)
                                   for a in parts])
            assert blob.shape[0] == BLOB_LEN, (blob.shape, BLOB_LEN)
            blob_cache[key_] = blob
        in_maps.append({"blob": blob_cache[key_]})
    return in_maps


def run(trace=False, **inputs):
    in_maps = _prepare_in_maps(**inputs)
    nc = _get_nc()
    res = bass_utils.run_bass_kernel_spmd(
        nc, in_maps, core_ids=list(range(N_CORES)), trace=trace)
    full = np.empty((B, S, E), dtype=np.float32)
    for c in range(N_CORES):
        b, g = c // 2, c % 2
        # out rows are [qt, p, mm] blocked: s = qt*SQ + mm*128 + p
        o = res.results[c]["out"].astype(np.float32)
        full[b, :, g * HC:(g + 1) * HC] = o.transpose(0, 2, 1, 3).reshape(
            S, HC)
    return full, res


def kernel(**inputs) -> np.ndarray:
    full, _ = run(trace=False, **inputs)
    return full


def bench(n_iters=5, repeats=5, nc=None, **inputs):
    """Estimate on-device NEFF time: chain n_iters executions with a tiny
    data dependency (no CSE, strict serialization), time with device-resident
    inputs, and report the marginal per-iteration wall time."""
    import time
    import jax
    from jax.sharding import Mesh, PartitionSpec
    from jax.experimental.shard_map import shard_map
    import concourse.bass2jax as bass2jax
    import concourse.mybir as mb

    if nc is None:
        nc = _get_nc()
    in_maps = _prepare_in_maps(**inputs)
    bass2jax.install_neuronx_cc_hook()

    partition_name = nc.partition_id_tensor.name if nc.partition_id_tensor else None
    in_names, out_names, out_avals = [], [], []
    for alloc in nc.m.functions[0].allocations:
        if not isinstance(alloc, mb.MemoryLocationSet):
            continue
        name = alloc.memorylocations[0].name
        if alloc.kind == "ExternalInput":
            if name != partition_name:
                in_names.append(name)
        elif alloc.kind == "ExternalOutput":
            out_names.append(name)
            out_avals.append(
                jax.core.ShapedArray(tuple(alloc.tensor_shape),
                                     mb.dt.np(alloc.dtype)))
    n_params = len(in_names)
    all_in_names = list(in_names) + list(out_names)
    if partition_name is not None:
        all_in_names.append(partition_name)

    def _body(*args):
        operands = list(args)
        if partition_name is not None:
            operands.append(bass2jax.partition_id_tensor())
        outs = bass2jax._bass_exec_p.bind(
            *operands,
            out_avals=tuple(out_avals),
            in_names=tuple(all_in_names),
            out_names=tuple(out_names),
            lowering_input_output_aliases=(),
            sim_require_finite=True,
            sim_require_nnan=True,
            nc=nc)
        return tuple(outs)

    devices = jax.devices()[:N_CORES]
    mesh = Mesh(np.asarray(devices), ("core",))
    n_outs = len(out_names)
    in_specs = (PartitionSpec("core"),) * (n_params + n_outs)
    out_specs = (PartitionSpec("core"),) * n_outs

    per_core = [[np.asarray(m[name]) for name in in_names] for m in in_maps]
    concat_in = [np.concatenate([per_core[c][i] for c in range(N_CORES)], axis=0)
                 for i in range(n_params)]
    concat_zeros = [np.zeros((N_CORES * a.shape[0], *a.shape[1:]), a.dtype)
                    for a in out_avals]

    sharding = jax.sharding.NamedSharding(mesh, PartitionSpec("core"))
    dev_in = [jax.device_put(x, sharding) for x in concat_in + concat_zeros]

    # donate the output buffers and chain each call's outputs into the next
    # call's donated outputs: executions serialize on-device, memory stays
    # bounded, and M iterations aggregate enough device time to dominate the
    # ~100ms axon RTT quantum.
    donate = tuple(range(n_params, n_params + n_outs))
    fn = jax.jit(shard_map(_body, mesh=mesh, in_specs=in_specs,
                           out_specs=out_specs, check_rep=False),
                 keep_unused=True, donate_argnums=donate)
    params = dev_in[:n_params]
    outs = tuple(dev_in[n_params:])
    outs = fn(*params, *outs)  # warm
    jax.block_until_ready(outs)

    def run_m(m):
        nonlocal outs
        t0 = time.perf_counter()
        for _ in range(m):
            outs = fn(*params, *outs)
        jax.block_until_ready(outs)
        return time.perf_counter() - t0

    # The axon dispatch path is bimodal (~48ms vs ~84ms floor for a short
    # chain), so a two-point min-min marginal can be wildly off when the two
    # points land in different states. Use the median at each of three chain
    # lengths and a least-squares slope instead.
    ms = [8, 8 + n_iters // 2, 8 + n_iters]
    med = {}
    for m in ms:
        ts = sorted(run_m(m) for _ in range(repeats))
        med[m] = ts[len(ts) // 2]
    mean_m = sum(ms) / len(ms)
    mean_t = sum(med.values()) / len(ms)
    slope = (sum((m - mean_m) * (med[m] - mean_t) for m in ms)
             / sum((m - mean_m) ** 2 for m in ms))
    return slope * 1e9, {m: med[m] for m in ms}

